# revision 36
# baseline (speedup 1.0000x reference)
"""VQ-codebook 3x3 conv (nn_CConv) on 8 Trainium2 NeuronCores.

Sharding: data-parallel over the batch (16 images -> 2 per core); the small
codebook-derived weights / scales / bias are replicated to every core.
Host-side work is layout only: batch split, reshape/transpose of the index
and scale matrices, and the codebook row gather (pure indexing, no
arithmetic).

Per-core device program (one NEFF, SPMD over 8 cores):
  - weight build (on device): fp16 round-trip of scales (dequant emulation),
    multiply by cut, then 9 per-tap [128in x 128out] multiplies of the
    k-major gathered codebook rows into 9 separate fp16 stationary tiles --
    per-tap tiles let the first conv matmuls start as soon as tap 0 is
    ready instead of waiting for the whole weight tensor.
  - conv: images are zero-padded to rows of PITCH 113 in SBUF: each row is
    [pad | x0..x111], so the left-pad zero of row r+1 doubles as the
    right-pad zero of row r (one junk column per row instead of two).
    The 3x3 conv is 9 accumulating PE matmuls over shifted views of the
    flattened padded image, fp16 in / fp32 PSUM accumulate. Junk outputs at
    w=112 of each row are computed but dropped by the strided output DMA.
  - images are processed in row-slabs; image 0 uses a graduated ramp
    (8,8,12,...) so the PE can start while the input-DMA pipeline fills,
    image 1 ends with a tiny 4-row slab to shorten the final-DMA tail.
    Input loads ride the sync HWDGE queue and are cast f32->f16 by the
    scalar engine; PSUM is evacuated with a fused per-partition bias add on
    the vector engine into fp16 (host widens the output to f32, lossless);
    output DMAs ride the scalar queue except the last two slabs, which use
    the (by-then idle) sync queue to dodge head-of-line blocking.
  - warm-up matmuls run during the prologue so the HAM clock gate reaches
    2.4 GHz before the real matmuls start; any PE idle gap makes HAM
    re-throttle to half rate for a ~3.4us window, so the warmup count is
    sized to end right as slab 0 and the weights become ready.
"""
import sys
import types
from contextlib import ExitStack

import numpy as np

import concourse.tile as tile
from concourse import bacc, mybir


def _ensure_axon_hooks_module():
    """This image's antenv package lacks axon_hooks; bass_utils imports it
    when tracing is requested (e.g. BASS_TRACE=1). Provide a no-op shim."""
    try:
        import antenv

        if "antenv.axon_hooks" not in sys.modules and not hasattr(
            antenv, "axon_hooks"
        ):
            mod = types.ModuleType("antenv.axon_hooks")
            holder = [None]
            mod.set_axon_ntff_profile_hook = lambda h: holder.__setitem__(0, h)
            mod.get_axon_ntff_profile_hook = lambda: holder[0]
            antenv.axon_hooks = mod
            sys.modules["antenv.axon_hooks"] = mod
    except Exception:
        pass


_ensure_axon_hooks_module()

from concourse import bass_utils  # noqa: E402

P = 128
H = W = 112
WP = 113
IMGS = 2
N_CORES = 8

f32 = mybir.dt.float32
f16 = mybir.dt.float16

SLAB_PLAN = {0: [8, 8, 12, 16, 20, 24, 24], 1: [28, 28, 24, 16, 10, 4, 2]}
MAX_SO = 28
WARM_MMS = 17
SPACK = P + P + 1  # scalesT | cutT | bias

_CACHE = {}


def _slab_tiles(slab_out):
    n_pos = slab_out * WP - 1
    full, r = divmod(n_pos, 512)
    tiles = [512] * full
    if r:
        if r < 256 and full:
            tiles = [512] * (full - 1) + [(512 + r) // 2, (512 + r) - (512 + r) // 2]
        else:
            tiles.append(r)
    assert sum(tiles) == n_pos
    return tiles


def _build():
    nc = bacc.Bacc("TRN2", target_bir_lowering=False, debug=False)

    x_t = nc.dram_tensor("x", [IMGS, P, H, W], f32, kind="ExternalInput")
    # k-major codebook gather wraw[i, k*128 + o] = centroids[idx[o, i], k],
    # split so taps 0-2 (needed first by the PE) land before the slab-0 image
    # rows, and taps 3-8 trail them
    wrawA_t = nc.dram_tensor("wrawA", [P, 3 * P], f32, kind="ExternalInput")
    wrawB_t = nc.dram_tensor("wrawB", [P, 6 * P], f32, kind="ExternalInput")
    # small scale-side pack = one DMA: [scalesT | cutT | bias]
    spack_t = nc.dram_tensor("spack", [P, SPACK], f32, kind="ExternalInput")
    # fp16 output: halves the store traffic; host widens to f32 (lossless)
    out_t = nc.dram_tensor("out", [IMGS, P, H, W], f16, kind="ExternalOutput")

    with tile.TileContext(nc) as tc, ExitStack() as ctx:
        wb = ctx.enter_context(tc.tile_pool(name="wb", bufs=1))
        xp = ctx.enter_context(tc.tile_pool(name="xp", bufs=4))
        op = ctx.enter_context(tc.tile_pool(name="op", bufs=4))
        ps = ctx.enter_context(tc.tile_pool(name="ps", bufs=6, space="PSUM"))
        xs = ctx.enter_context(tc.tile_pool(name="xs", bufs=4))

        # ALL prologue loads ride the sync queue: completions land in rough
        # FIFO order, each ~its own transfer time after the previous, whereas
        # a second queue's packets get starved by per-packet round-robin
        # against big transfers. Order = (consumer-latency-aware): spack
        # (feeds the scc16 chain), taps 0-2, the slab-0 rows, taps 3-8.
        spack = wb.tile([P, SPACK], f32, tag="spack")
        nc.sync.dma_start(spack[:], spack_t.ap())
        bias_s = spack[:, 2 * P:2 * P + 1]
        w_rawA = wb.tile([P, 3 * P], f32, tag="w_rawA")
        nc.sync.dma_start(w_rawA[:], wrawA_t.ap())

        so0 = SLAB_PLAN[0][0]
        nrows0 = min(H, so0 + 1)
        pre_stage = xs.tile([P, (MAX_SO + 2) * W], f32, tag="xstage")
        nc.sync.dma_start(pre_stage[:, :nrows0 * W], x_t.ap()[0, :, 0:nrows0, :])
        w_rawB = wb.tile([P, 6 * P], f32, tag="w_rawB")
        nc.sync.dma_start(w_rawB[:], wrawB_t.ap())

        # PE warmup (HAM clock ramp); warm tile memset on the vector engine
        wrm = wb.tile([P, 512], f16, tag="warm")
        nc.vector.memset(wrm[:], 0.0)
        pw = ps.tile([P, 512], f32, tag="pst")
        for _ in range(WARM_MMS):
            nc.tensor.matmul(pw[:], wrm[:, :P], wrm[:], start=True, stop=True)

        # ---- weight build (all-fp16 fast path on the vector engine) ----
        # sc16 IS the reference's fp16-roundtripped scales (dequant emulation)
        sc16 = wb.tile([P, P], f16, tag="sc16")
        nc.vector.tensor_copy(sc16[:], spack[:, 0:P])
        cut16 = wb.tile([P, P], f16, tag="cut16")
        nc.vector.tensor_copy(cut16[:], spack[:, P:2 * P])
        scc16 = wb.tile([P, P], f16, tag="scc16")
        nc.vector.tensor_tensor(
            out=scc16[:], in0=sc16[:], in1=cut16[:], op=mybir.AluOpType.mult
        )
        # per-tap stationary tiles: w_k[k][i, o] = wraw[i, k, o] * scc[i, o];
        # fp16 2x-mode multiplies produce taps faster than the PE consumes them
        wraw16a = wb.tile([P, 3 * P], f16, tag="wraw16a")
        nc.vector.tensor_copy(wraw16a[:], w_rawA[:])
        w_k = []
        for k in range(3):
            wk = wb.tile([P, P], f16, tag=f"w_k{k}")
            nc.vector.tensor_tensor(
                out=wk[:], in0=wraw16a[:, k * P:(k + 1) * P], in1=scc16[:],
                op=mybir.AluOpType.mult,
            )
            w_k.append(wk)
        # taps 3-8 multiply the f32 rows directly (f32xf32->f16): each tap is
        # ready ~290ns after wrawB lands instead of waiting for a bulk cast
        sc32 = wb.tile([P, P], f32, tag="sc32")
        nc.vector.tensor_copy(sc32[:], sc16[:])
        scc32 = wb.tile([P, P], f32, tag="scc32")
        nc.vector.tensor_tensor(
            out=scc32[:], in0=sc32[:], in1=spack[:, P:2 * P],
            op=mybir.AluOpType.mult,
        )
        for k in range(3, 9):
            wk = wb.tile([P, P], f16, tag=f"w_k{k}")
            nc.vector.tensor_tensor(
                out=wk[:], in0=w_rawB[:, (k - 3) * P:(k - 2) * P], in1=scc32[:],
                op=mybir.AluOpType.mult,
            )
            w_k.append(wk)

        # slab-0 padded tile built eagerly, cast on the VECTOR engine: the
        # scalar-engine path was measured to wake ~1.4us after the input
        # landed, and the DVE slots this in by readiness between weight ops
        max_xpad_len = (MAX_SO + 2) * WP + 1
        so0_in = so0 + 2
        xpad0 = xp.tile([P, max_xpad_len], f16, tag="xpad")
        xpad0_3 = xpad0[:, :so0_in * WP].rearrange("p (r c) -> p r c", c=WP)
        nc.gpsimd.memset(xpad0_3[:, :, 0:1], 0.0)
        nc.gpsimd.memset(xpad0[:, so0_in * WP:so0_in * WP + 1], 0.0)
        nc.gpsimd.memset(xpad0[:, 0:WP], 0.0)
        nc.vector.tensor_copy(
            xpad0_3[:, 1:1 + nrows0, 1:1 + W],
            pre_stage[:, :nrows0 * W].rearrange("p (r c) -> p r c", c=W),
        )

        # ---- conv slabs ----
        max_oslab_len = MAX_SO * WP
        max_stage = (MAX_SO + 2) * W
        n_slabs_total = sum(len(v) for v in SLAB_PLAN.values())
        slab_idx = 0
        n_stage_dmas = 0
        for img in range(IMGS):
            h0 = 0
            for so in SLAB_PLAN[img]:
                slab_in = so + 2
                xpad_len = slab_in * WP + 1
                if img == 0 and h0 == 0:
                    xpad = xpad0  # built eagerly above
                else:
                    xpad = xp.tile([P, max_xpad_len], f16, tag="xpad")
                    xpad3 = xpad[:, :slab_in * WP].rearrange("p (r c) -> p r c", c=WP)
                    # zero borders: left-pad col of every row + trailing guard
                    # element (the right pad of the very last position)
                    nc.gpsimd.memset(xpad3[:, :, 0:1], 0.0)
                    nc.gpsimd.memset(xpad[:, xpad_len - 1:xpad_len], 0.0)
                    if h0 == 0:
                        nc.gpsimd.memset(xpad[:, 0:WP], 0.0)
                    elif h0 + so == H:
                        nc.gpsimd.memset(xpad[:, (slab_in - 1) * WP:xpad_len - 1], 0.0)
                    # interior rows: f32 staged load, scalar-engine cast to f16
                    r_lo = max(0, h0 - 1)
                    r_hi = min(H, h0 + so + 1)
                    j0 = r_lo - (h0 - 1)
                    nrows = r_hi - r_lo
                    if n_stage_dmas == 1:
                        # gate later bulk prefetches behind the slab-0 load:
                        # concurrent in-flight DMAs smear each other's
                        # completion (packets interleave across the 16 SDMA
                        # engines), so this dummy's issue-side wait keeps the
                        # critical prologue transfers clean
                        gate = wb.tile([P, 1], f32, tag="gate")
                        nc.sync.dma_start(gate[:], pre_stage[:, 0:1])
                    stage = xs.tile([P, max_stage], f32, tag="xstage")
                    nc.sync.dma_start(
                        stage[:, :nrows * W], x_t.ap()[img, :, r_lo:r_hi, :]
                    )
                    n_stage_dmas += 1
                    nc.scalar.copy(
                        xpad3[:, j0:j0 + nrows, 1:1 + W],
                        stage[:, :nrows * W].rearrange("p (r c) -> p r c", c=W),
                    )

                oslab = op.tile([P, max_oslab_len], f16, tag="oslab")
                q0 = 0
                for n in _slab_tiles(so):
                    pst = ps.tile([P, 512], f32, tag="pst")
                    for k in range(9):
                        dh, dw = divmod(k, 3)
                        off = q0 + dh * WP + dw
                        nc.tensor.matmul(
                            pst[:, :n],
                            w_k[k][:],
                            xpad[:, off:off + n],
                            start=(k == 0),
                            stop=(k == 8),
                        )
                    nc.vector.tensor_scalar_add(
                        oslab[:, q0:q0 + n], pst[:, :n], bias_s
                    )
                    q0 += n

                osrc = oslab[:, :so * WP].rearrange("p (r c) -> p r c", c=WP)[:, :, 0:W]
                # alternate queues for the tapering final slabs so each of
                # the last DMAs issues into an empty ring (issuing behind a
                # big in-flight transfer blocks ~3us on ring credits) and
                # their completions overlap the remaining compute
                if slab_idx in (n_slabs_total - 2, n_slabs_total - 4):
                    nc.sync.dma_start(out_t.ap()[img, :, h0:h0 + so, :], osrc)
                else:
                    nc.scalar.dma_start(out_t.ap()[img, :, h0:h0 + so, :], osrc)
                h0 += so
                slab_idx += 1

    nc.compile()
    return nc


def _make_in_maps(inputs):
    x = np.ascontiguousarray(np.asarray(inputs["x"], dtype=np.float32))
    cent = np.asarray(inputs["centroids"], dtype=np.float32).reshape(512, 9)
    idxT = np.asarray(inputs["idx"]).reshape(P, P).T          # [i, o]
    scalesT = np.ascontiguousarray(
        np.asarray(inputs["scales"], dtype=np.float32).reshape(P, P).T
    )
    cutT = np.ascontiguousarray(
        np.asarray(inputs["cut"], dtype=np.float32).reshape(P, P).T
    )
    bias = np.ascontiguousarray(
        np.asarray(inputs["bias"], dtype=np.float32).reshape(P, 1)
    )
    # [i, o, k] -> k-major [i, k, o] (pure layout), split taps 0-2 / 3-8
    wraw_km = cent[idxT].transpose(0, 2, 1)
    wrawA = np.ascontiguousarray(wraw_km[:, :3, :].reshape(P, 3 * P))
    wrawB = np.ascontiguousarray(wraw_km[:, 3:, :].reshape(P, 6 * P))
    spack = np.ascontiguousarray(
        np.concatenate([scalesT, cutT, bias], axis=1, dtype=np.float32)
    )

    base = {"wrawA": wrawA, "wrawB": wrawB, "spack": spack}
    maps = []
    for c in range(N_CORES):
        m = dict(base)
        m["x"] = np.ascontiguousarray(x[IMGS * c:IMGS * (c + 1)])
        maps.append(m)
    return maps


def _get_nc():
    if "nc" not in _CACHE:
        _CACHE["nc"] = _build()
    return _CACHE["nc"]


def _run(inputs, trace=False):
    nc = _get_nc()
    in_maps = _make_in_maps(inputs)
    res = bass_utils.run_bass_kernel_spmd(
        nc, in_maps, core_ids=list(range(N_CORES)), trace=trace
    )
    out = np.concatenate([res.results[c]["out"] for c in range(N_CORES)], axis=0)
    out = out.astype(np.float32)  # widen fp16 device output (lossless)
    return out, res


def kernel(**inputs) -> np.ndarray:
    out, _ = _run(inputs, trace=False)
    return out


# revision 37
# speedup vs baseline: 1.0542x; 1.0542x over previous
"""VQ-codebook 3x3 conv (nn_CConv) on 8 Trainium2 NeuronCores.

Sharding: data-parallel over the batch (16 images -> 2 per core); the small
codebook-derived weights / scales / bias are replicated to every core.
Host-side work is layout only: batch split, reshape/transpose of the index
and scale matrices, and the codebook row gather (pure indexing, no
arithmetic).

Per-core device program (one NEFF, SPMD over 8 cores):
  - weight build (on device): fp16 round-trip of scales (dequant emulation),
    multiply by cut, then 9 per-tap [128in x 128out] multiplies of the
    k-major gathered codebook rows into 9 separate fp16 stationary tiles --
    per-tap tiles let the first conv matmuls start as soon as tap 0 is
    ready instead of waiting for the whole weight tensor.
  - conv: images are zero-padded to rows of PITCH 113 in SBUF: each row is
    [pad | x0..x111], so the left-pad zero of row r+1 doubles as the
    right-pad zero of row r (one junk column per row instead of two).
    The 3x3 conv is 9 accumulating PE matmuls over shifted views of the
    flattened padded image, fp16 in / fp32 PSUM accumulate. Junk outputs at
    w=112 of each row are computed but dropped by the strided output DMA.
  - images are processed in row-slabs; image 0 uses a graduated ramp
    (8,8,12,...) so the PE can start while the input-DMA pipeline fills,
    image 1 ends with a tiny 4-row slab to shorten the final-DMA tail.
    Input loads ride the sync HWDGE queue and are cast f32->f16 by the
    scalar engine; PSUM is evacuated with a fused per-partition bias add on
    the vector engine into fp16 (host widens the output to f32, lossless);
    output DMAs ride the scalar queue except the last two slabs, which use
    the (by-then idle) sync queue to dodge head-of-line blocking.
  - warm-up matmuls run during the prologue so the HAM clock gate reaches
    2.4 GHz before the real matmuls start; any PE idle gap makes HAM
    re-throttle to half rate for a ~3.4us window, so the warmup count is
    sized to end right as slab 0 and the weights become ready.
"""
import sys
import types
from contextlib import ExitStack

import numpy as np

import concourse.tile as tile
from concourse import bacc, mybir


def _ensure_axon_hooks_module():
    """This image's antenv package lacks axon_hooks; bass_utils imports it
    when tracing is requested (e.g. BASS_TRACE=1). Provide a no-op shim."""
    try:
        import antenv

        if "antenv.axon_hooks" not in sys.modules and not hasattr(
            antenv, "axon_hooks"
        ):
            mod = types.ModuleType("antenv.axon_hooks")
            holder = [None]
            mod.set_axon_ntff_profile_hook = lambda h: holder.__setitem__(0, h)
            mod.get_axon_ntff_profile_hook = lambda: holder[0]
            antenv.axon_hooks = mod
            sys.modules["antenv.axon_hooks"] = mod
    except Exception:
        pass


_ensure_axon_hooks_module()

from concourse import bass_utils  # noqa: E402

P = 128
H = W = 112
WP = 113
IMGS = 2
N_CORES = 8

f32 = mybir.dt.float32
f16 = mybir.dt.float16

SLAB_PLAN = {0: [8, 8, 12, 16, 20, 24, 24], 1: [28, 28, 24, 16, 10, 4, 2]}
MAX_SO = 28
WARM_MMS = 16
SPACK = P + P + 1  # scalesT | cutT | bias

_CACHE = {}


def _slab_tiles(slab_out):
    n_pos = slab_out * WP - 1
    full, r = divmod(n_pos, 512)
    tiles = [512] * full
    if r:
        if r < 256 and full:
            tiles = [512] * (full - 1) + [(512 + r) // 2, (512 + r) - (512 + r) // 2]
        else:
            tiles.append(r)
    assert sum(tiles) == n_pos
    return tiles


def _build():
    nc = bacc.Bacc("TRN2", target_bir_lowering=False, debug=False)

    x_t = nc.dram_tensor("x", [IMGS, P, H, W], f32, kind="ExternalInput")
    # k-major codebook gather wraw[i, k*128 + o] = centroids[idx[o, i], k],
    # split so taps 0-2 (needed first by the PE) land before the slab-0 image
    # rows, and taps 3-8 trail them
    wrawA_t = nc.dram_tensor("wrawA", [P, 3 * P], f32, kind="ExternalInput")
    wrawB_t = nc.dram_tensor("wrawB", [P, 6 * P], f32, kind="ExternalInput")
    # small scale-side pack = one DMA: [scalesT | cutT | bias]
    spack_t = nc.dram_tensor("spack", [P, SPACK], f32, kind="ExternalInput")
    # fp16 output: halves the store traffic; host widens to f32 (lossless)
    out_t = nc.dram_tensor("out", [IMGS, P, H, W], f16, kind="ExternalOutput")

    with tile.TileContext(nc) as tc, ExitStack() as ctx:
        wb = ctx.enter_context(tc.tile_pool(name="wb", bufs=1))
        xp = ctx.enter_context(tc.tile_pool(name="xp", bufs=4))
        op = ctx.enter_context(tc.tile_pool(name="op", bufs=4))
        ps = ctx.enter_context(tc.tile_pool(name="ps", bufs=6, space="PSUM"))
        xs = ctx.enter_context(tc.tile_pool(name="xs", bufs=4))

        # ALL prologue loads ride the sync queue: completions land in rough
        # FIFO order, each ~its own transfer time after the previous, whereas
        # a second queue's packets get starved by per-packet round-robin
        # against big transfers. Order = (consumer-latency-aware): spack
        # (feeds the scc16 chain), taps 0-2, the slab-0 rows, taps 3-8.
        spack = wb.tile([P, SPACK], f32, tag="spack")
        nc.sync.dma_start(spack[:], spack_t.ap())
        bias_s = spack[:, 2 * P:2 * P + 1]
        w_rawA = wb.tile([P, 3 * P], f32, tag="w_rawA")
        nc.sync.dma_start(w_rawA[:], wrawA_t.ap())

        so0 = SLAB_PLAN[0][0]
        nrows0 = min(H, so0 + 1)
        pre_stage = xs.tile([P, (MAX_SO + 2) * W], f32, tag="xstage")
        nc.sync.dma_start(pre_stage[:, :nrows0 * W], x_t.ap()[0, :, 0:nrows0, :])
        w_rawB = wb.tile([P, 6 * P], f32, tag="w_rawB")
        nc.sync.dma_start(w_rawB[:], wrawB_t.ap())

        # PE warmup (HAM clock ramp); warm tile memset on the vector engine
        wrm = wb.tile([P, 512], f16, tag="warm")
        nc.vector.memset(wrm[:], 0.0)
        pw = ps.tile([P, 512], f32, tag="pst")
        for _ in range(WARM_MMS):
            nc.tensor.matmul(pw[:], wrm[:, :P], wrm[:], start=True, stop=True)

        # ---- weight build (all-fp16 fast path on the vector engine) ----
        # sc16 IS the reference's fp16-roundtripped scales (dequant emulation)
        sc16 = wb.tile([P, P], f16, tag="sc16")
        nc.vector.tensor_copy(sc16[:], spack[:, 0:P])
        cut16 = wb.tile([P, P], f16, tag="cut16")
        nc.vector.tensor_copy(cut16[:], spack[:, P:2 * P])
        scc16 = wb.tile([P, P], f16, tag="scc16")
        nc.vector.tensor_tensor(
            out=scc16[:], in0=sc16[:], in1=cut16[:], op=mybir.AluOpType.mult
        )
        # per-tap stationary tiles: w_k[k][i, o] = wraw[i, k, o] * scc[i, o];
        # fp16 2x-mode multiplies produce taps faster than the PE consumes them
        wraw16a = wb.tile([P, 3 * P], f16, tag="wraw16a")
        nc.vector.tensor_copy(wraw16a[:], w_rawA[:])
        w_k = []
        for k in range(3):
            wk = wb.tile([P, P], f16, tag=f"w_k{k}")
            nc.vector.tensor_tensor(
                out=wk[:], in0=wraw16a[:, k * P:(k + 1) * P], in1=scc16[:],
                op=mybir.AluOpType.mult,
            )
            w_k.append(wk)
        wraw16b = wb.tile([P, 6 * P], f16, tag="wraw16b")
        nc.vector.tensor_copy(wraw16b[:], w_rawB[:])
        for k in range(3, 9):
            wk = wb.tile([P, P], f16, tag=f"w_k{k}")
            nc.vector.tensor_tensor(
                out=wk[:], in0=wraw16b[:, (k - 3) * P:(k - 2) * P], in1=scc16[:],
                op=mybir.AluOpType.mult,
            )
            w_k.append(wk)

        # slab-0 padded tile built eagerly, cast on the VECTOR engine: the
        # scalar-engine path was measured to wake ~1.4us after the input
        # landed, and the DVE slots this in by readiness between weight ops
        max_xpad_len = (MAX_SO + 2) * WP + 1
        so0_in = so0 + 2
        xpad0 = xp.tile([P, max_xpad_len], f16, tag="xpad")
        xpad0_3 = xpad0[:, :so0_in * WP].rearrange("p (r c) -> p r c", c=WP)
        nc.gpsimd.memset(xpad0_3[:, :, 0:1], 0.0)
        nc.gpsimd.memset(xpad0[:, so0_in * WP:so0_in * WP + 1], 0.0)
        nc.gpsimd.memset(xpad0[:, 0:WP], 0.0)
        nc.vector.tensor_copy(
            xpad0_3[:, 1:1 + nrows0, 1:1 + W],
            pre_stage[:, :nrows0 * W].rearrange("p (r c) -> p r c", c=W),
        )

        # ---- conv slabs ----
        max_oslab_len = MAX_SO * WP
        max_stage = (MAX_SO + 2) * W
        n_slabs_total = sum(len(v) for v in SLAB_PLAN.values())
        slab_idx = 0
        n_stage_dmas = 0
        for img in range(IMGS):
            h0 = 0
            for so in SLAB_PLAN[img]:
                slab_in = so + 2
                xpad_len = slab_in * WP + 1
                if img == 0 and h0 == 0:
                    xpad = xpad0  # built eagerly above
                else:
                    xpad = xp.tile([P, max_xpad_len], f16, tag="xpad")
                    xpad3 = xpad[:, :slab_in * WP].rearrange("p (r c) -> p r c", c=WP)
                    # zero borders: left-pad col of every row + trailing guard
                    # element (the right pad of the very last position)
                    nc.gpsimd.memset(xpad3[:, :, 0:1], 0.0)
                    nc.gpsimd.memset(xpad[:, xpad_len - 1:xpad_len], 0.0)
                    if h0 == 0:
                        nc.gpsimd.memset(xpad[:, 0:WP], 0.0)
                    elif h0 + so == H:
                        nc.gpsimd.memset(xpad[:, (slab_in - 1) * WP:xpad_len - 1], 0.0)
                    # interior rows: f32 staged load, scalar-engine cast to f16
                    r_lo = max(0, h0 - 1)
                    r_hi = min(H, h0 + so + 1)
                    j0 = r_lo - (h0 - 1)
                    nrows = r_hi - r_lo
                    if n_stage_dmas == 1:
                        # gate later bulk prefetches behind the slab-0 load:
                        # concurrent in-flight DMAs smear each other's
                        # completion (packets interleave across the 16 SDMA
                        # engines), so this dummy's issue-side wait keeps the
                        # critical prologue transfers clean
                        gate = wb.tile([P, 1], f32, tag="gate")
                        nc.sync.dma_start(gate[:], pre_stage[:, 0:1])
                    stage = xs.tile([P, max_stage], f32, tag="xstage")
                    nc.sync.dma_start(
                        stage[:, :nrows * W], x_t.ap()[img, :, r_lo:r_hi, :]
                    )
                    n_stage_dmas += 1
                    nc.scalar.copy(
                        xpad3[:, j0:j0 + nrows, 1:1 + W],
                        stage[:, :nrows * W].rearrange("p (r c) -> p r c", c=W),
                    )

                oslab = op.tile([P, max_oslab_len], f16, tag="oslab")
                q0 = 0
                for n in _slab_tiles(so):
                    pst = ps.tile([P, 512], f32, tag="pst")
                    for k in range(9):
                        dh, dw = divmod(k, 3)
                        off = q0 + dh * WP + dw
                        nc.tensor.matmul(
                            pst[:, :n],
                            w_k[k][:],
                            xpad[:, off:off + n],
                            start=(k == 0),
                            stop=(k == 8),
                        )
                    nc.vector.tensor_scalar_add(
                        oslab[:, q0:q0 + n], pst[:, :n], bias_s
                    )
                    q0 += n

                osrc = oslab[:, :so * WP].rearrange("p (r c) -> p r c", c=WP)[:, :, 0:W]
                # alternate queues for the tapering final slabs so each of
                # the last DMAs issues into an empty ring (issuing behind a
                # big in-flight transfer blocks ~3us on ring credits) and
                # their completions overlap the remaining compute
                if slab_idx in (n_slabs_total - 2, n_slabs_total - 4):
                    nc.sync.dma_start(out_t.ap()[img, :, h0:h0 + so, :], osrc)
                else:
                    nc.scalar.dma_start(out_t.ap()[img, :, h0:h0 + so, :], osrc)
                h0 += so
                slab_idx += 1

    nc.compile()
    return nc


def _make_in_maps(inputs):
    x = np.ascontiguousarray(np.asarray(inputs["x"], dtype=np.float32))
    cent = np.asarray(inputs["centroids"], dtype=np.float32).reshape(512, 9)
    idxT = np.asarray(inputs["idx"]).reshape(P, P).T          # [i, o]
    scalesT = np.ascontiguousarray(
        np.asarray(inputs["scales"], dtype=np.float32).reshape(P, P).T
    )
    cutT = np.ascontiguousarray(
        np.asarray(inputs["cut"], dtype=np.float32).reshape(P, P).T
    )
    bias = np.ascontiguousarray(
        np.asarray(inputs["bias"], dtype=np.float32).reshape(P, 1)
    )
    # [i, o, k] -> k-major [i, k, o] (pure layout), split taps 0-2 / 3-8
    wraw_km = cent[idxT].transpose(0, 2, 1)
    wrawA = np.ascontiguousarray(wraw_km[:, :3, :].reshape(P, 3 * P))
    wrawB = np.ascontiguousarray(wraw_km[:, 3:, :].reshape(P, 6 * P))
    spack = np.ascontiguousarray(
        np.concatenate([scalesT, cutT, bias], axis=1, dtype=np.float32)
    )

    base = {"wrawA": wrawA, "wrawB": wrawB, "spack": spack}
    maps = []
    for c in range(N_CORES):
        m = dict(base)
        m["x"] = np.ascontiguousarray(x[IMGS * c:IMGS * (c + 1)])
        maps.append(m)
    return maps


def _get_nc():
    if "nc" not in _CACHE:
        _CACHE["nc"] = _build()
    return _CACHE["nc"]


def _run(inputs, trace=False):
    nc = _get_nc()
    in_maps = _make_in_maps(inputs)
    res = bass_utils.run_bass_kernel_spmd(
        nc, in_maps, core_ids=list(range(N_CORES)), trace=trace
    )
    out = np.concatenate([res.results[c]["out"] for c in range(N_CORES)], axis=0)
    out = out.astype(np.float32)  # widen fp16 device output (lossless)
    return out, res


def kernel(**inputs) -> np.ndarray:
    out, _ = _run(inputs, trace=False)
    return out
